# revision 2
# baseline (speedup 1.0000x reference)
# Trainium2 Bass kernel for nn_ExpandFrame: gaussian-upsampling attention
#   e = cumsum(duration, -1); c = e - 0.5*round(duration)
#   logits[b,n,t] = temp * (t - c[b,n])^2 ;  temp = -1/(5*sqrt(duration[0,0]))
#   w = softmax(logits, axis=n) ;  out[b,d,t] = sum_n w[b,n,t] * hidden[b,n,d]
#
# Strategy: data-parallel over batch B=16 across 8 cores (2 batches/core).
# The weights form a narrow band (|t - c_n| <~ 30), so everything runs over
# host-computed static n-windows (128-aligned), shared across batches.
#
# Unlike the transpose-based variant, softmax runs directly in
# [n_partition, t_free] layout:
#   pos[n,t]  = Square(s*t + (-s*c[n]))        (scalar act, per-partition bias)
#   p[n,t]    = Exp(-pos)  (bf16)              (scalar act)
#   S[t]      = ones^T @ p                     (PE matmul, M=1)
#   r[t]      = 1/S                            (DVE reciprocal)
#   out[d,t] += hid[n,d]^T @ p[n,t]            (PE banded matmul, accumulate)
#   osb       = psum * r_bcast                 (DVE evac-multiply, normalizes)
# This removes all PE transposes, diag builds and wT copies; hidden is cast
# f32->bf16 during the SWDGE DMA load (separate queue from output writes).
# Columns t beyond the last center (cumsum < T) get a host-computed shift
# m[t] = max(0, pos_nearest - 40) subtracted before Exp so S never hits 0.
import numpy as np

B, N, D, T = 16, 1024, 1024, 4096
NCORES = 8
BPC = B // NCORES        # batches per core
P = 128                  # partitions
TT = 512                 # t-tile (PSUM bank = 512 fp32)
NTT = T // TT            # 8
KN = N // P              # 8 n-chunks
NDC = D // P             # 8 d-chunks
HGRP = 4                 # t-tiles per output DMA group (4*512*4B = 8KB rows)


def _host_prep(duration):
    """Centers, temp, static band windows, and tail stabilizer rows."""
    dur = np.asarray(duration, dtype=np.float32)
    e = np.cumsum(dur, axis=-1, dtype=np.float32)
    c = (e - np.float32(0.5) * np.round(dur)).astype(np.float32)   # [B, N]
    d00 = float(dur[0, 0])
    temp = -1.0 / (5.0 * np.sqrt(d00))
    s = float(np.sqrt(-temp))
    margin = int(np.ceil(np.sqrt(60.0 / -temp))) + 2

    lo = np.empty((B, NTT), dtype=np.int64)
    hi = np.empty((B, NTT), dtype=np.int64)
    t0s = np.arange(NTT) * TT
    for b in range(B):
        lo[b] = np.searchsorted(c[b], t0s - margin, side="left")
        hi[b] = np.searchsorted(c[b], t0s + (TT - 1) + margin, side="right")
    ulo = np.minimum(lo.min(axis=0), N - 1)
    uhi = np.maximum(hi.max(axis=0), ulo + 1)
    klo = ulo // P
    khi = np.minimum((uhi + P - 1) // P, KN)
    khi = np.maximum(khi, klo + 1)

    # Stabilizer: m[t] = max(0, s^2*dist_nearest^2 - 40), per batch; a tile
    # needs the subtract pass if ANY batch has m > 0 there (shared program).
    tgrid = np.arange(T, dtype=np.float64)
    msub = np.zeros((B, T), dtype=np.float32)
    for b in range(B):
        idx = np.searchsorted(c[b], tgrid)
        dl = np.abs(tgrid - c[b][np.clip(idx - 1, 0, N - 1)])
        dr = np.abs(c[b][np.clip(idx, 0, N - 1)] - tgrid)
        dmin = np.minimum(dl, dr)
        msub[b] = np.maximum((s * s) * (dmin * dmin) - 40.0, 0.0)
    need_m = (msub.reshape(B, NTT, TT).max(axis=2) > 0.0).any(axis=0)

    # c transposed per batch: cbt[b][p, k] = c[b, k*128 + p]
    cbt = np.ascontiguousarray(
        c.reshape(B, KN, P).transpose(0, 2, 1)).astype(np.float32)
    return cbt, s, klo, khi, need_m, msub


def _build(nc, klo, khi, need_m, s):
    import concourse.tile as tile
    import concourse.mybir as mybir

    f32 = mybir.dt.float32
    bf16 = mybir.dt.bfloat16
    i32 = mybir.dt.int32
    AF = mybir.ActivationFunctionType
    ALU = mybir.AluOpType

    hid = nc.dram_tensor("hidden", [BPC, N, D], f32, kind="ExternalInput").ap()
    cbt = nc.dram_tensor("cbt", [BPC, P, KN], f32, kind="ExternalInput").ap()
    msb = nc.dram_tensor("msub", [BPC, T], f32, kind="ExternalInput").ap()
    outd = nc.dram_tensor("out", [BPC, D, T], f32, kind="ExternalOutput").ap()

    kws = [int(khi[t] - klo[t]) for t in range(NTT)]
    off = [0]
    for t in range(NTT):
        off.append(off[-1] + kws[t])
    SKW = off[-1]
    any_m = bool(need_m.any())

    with tile.TileContext(nc) as tc:
        import contextlib
        with contextlib.ExitStack() as ctx:
            constp = ctx.enter_context(tc.tile_pool(name="const", bufs=1))
            cbp = ctx.enter_context(tc.tile_pool(name="cb", bufs=2))
            hidp = ctx.enter_context(tc.tile_pool(name="hid", bufs=2))
            biasp = ctx.enter_context(tc.tile_pool(name="bias", bufs=2))
            posp = ctx.enter_context(tc.tile_pool(name="pos", bufs=4))
            pap = ctx.enter_context(tc.tile_pool(name="pall", bufs=2))
            rsp = ctx.enter_context(tc.tile_pool(name="rs", bufs=2))
            rbp = ctx.enter_context(tc.tile_pool(name="rb", bufs=2))
            msp = ctx.enter_context(tc.tile_pool(name="ms", bufs=2))
            osbp = ctx.enter_context(tc.tile_pool(name="osb", bufs=3))
            pop = ctx.enter_context(tc.tile_pool(name="po", bufs=6, space="PSUM"))
            ssp = ctx.enter_context(tc.tile_pool(name="ss", bufs=2, space="PSUM"))

            # constants: t-iota row (same for every partition) and ones column
            trow_i = constp.tile([P, TT], i32)
            nc.gpsimd.iota(trow_i[:], pattern=[[1, TT]], base=0,
                           channel_multiplier=0)
            trow = constp.tile([P, TT], f32)
            nc.scalar.mul(trow[:], trow_i[:], 1.0)
            ones = constp.tile([P, 1], bf16)
            nc.gpsimd.memset(ones[:], 1.0)
            # warm the ACT spline tables before the DMA flood
            warm = constp.tile([P, 1], f32)
            nc.scalar.activation(warm[:], trow[:, 0:1], AF.Square,
                                 bias=0.0, scale=1.0)
            nc.scalar.activation(warm[:], warm[:], AF.Exp,
                                 bias=0.0, scale=-1.0)

            # prologue: small loads for both batches land before the writes
            cbs = []
            mrows = []
            for b in range(BPC):
                cb_sb = cbp.tile([P, KN], f32, tag="cb")
                nc.sync.dma_start(cb_sb[:], cbt[b])
                cbs.append(cb_sb)
                if any_m:
                    row = msp.tile([1, T], f32, tag="mrow")
                    nc.sync.dma_start(row[:], msb[b][None, :])
                    mrows.append(row)
                else:
                    mrows.append(None)

            for b in range(BPC):
                # hidden load, f32 -> bf16 cast in the DMA (SWDGE queue)
                hid_sb = hidp.tile([P, KN, D], bf16, tag="hid")
                for hk in range(2):
                    ks = hk * (KN // 2)
                    src = hid[b, ks * P:(ks + KN // 2) * P, :]
                    nc.gpsimd.dma_start(
                        hid_sb[:, ks:ks + KN // 2, :],
                        src.rearrange("(k p) d -> p k d", p=P))

                # bias_all[:, tt, k] = s*TT*tt - s*c[:, k]
                negsc = biasp.tile([P, KN], f32, tag="negsc")
                nc.vector.tensor_scalar_mul(negsc[:], cbs[b][:], -s)
                bias_all = biasp.tile([P, NTT, KN], f32, tag="bias")
                for tt in range(NTT):
                    nc.vector.tensor_scalar_add(bias_all[:, tt, :], negsc[:],
                                                float(s * TT * tt))

                mbc = {}
                for tt in range(NTT):
                    if need_m[tt]:
                        mt = msp.tile([P, TT], f32, tag="mb")
                        nc.gpsimd.partition_broadcast(
                            mt[:], mrows[b][:, tt * TT:(tt + 1) * TT])
                        mbc[tt] = mt

                # softmax in [n, t] layout + column sums via ones-matmul
                p_all = pap.tile([P, SKW, TT], bf16, tag="pall")
                r_sb = rsp.tile([1, T], f32, tag="rs")
                for tt in range(NTT):
                    kw = kws[tt]
                    s_ps = ssp.tile([1, TT], f32, tag="S")
                    for ki in range(kw):
                        k = int(klo[tt]) + ki
                        pos = posp.tile([P, TT], f32, tag="pos")
                        nc.scalar.activation(
                            pos[:], trow[:], AF.Square,
                            bias=bias_all[:, tt, k:k + 1], scale=s)
                        psl = p_all[:, off[tt] + ki, :]
                        if tt in mbc:
                            pos2 = posp.tile([P, TT], f32, tag="pos2")
                            nc.vector.tensor_tensor(
                                pos2[:], pos[:], mbc[tt][:], op=ALU.subtract)
                            pos = pos2
                        nc.scalar.activation(psl, pos[:], AF.Exp,
                                             bias=0.0, scale=-1.0)
                        nc.tensor.matmul(s_ps[:], ones[:], psl,
                                         start=(ki == 0), stop=(ki == kw - 1))
                    nc.vector.reciprocal(r_sb[:, tt * TT:(tt + 1) * TT],
                                         s_ps[:])

                # banded contraction, normalize on PSUM evacuation, store
                for h in range(NTT // HGRP):
                    rb = rbp.tile([P, HGRP * TT], f32, tag="rb")
                    nc.gpsimd.partition_broadcast(
                        rb[:], r_sb[:, h * HGRP * TT:(h + 1) * HGRP * TT])
                    for dci in range(NDC):
                        osb = osbp.tile([P, HGRP * TT], f32, tag="osb")
                        for j in range(HGRP):
                            tt = HGRP * h + j
                            kw = kws[tt]
                            po = pop.tile([P, TT], f32, tag="po")
                            for ki in range(kw):
                                k = int(klo[tt]) + ki
                                nc.tensor.matmul(
                                    po[:],
                                    hid_sb[:, k, dci * P:(dci + 1) * P],
                                    p_all[:, off[tt] + ki, :],
                                    start=(ki == 0), stop=(ki == kw - 1))
                            nc.vector.tensor_tensor(
                                osb[:, j * TT:(j + 1) * TT], po[:],
                                rb[:, j * TT:(j + 1) * TT], op=ALU.mult)
                        nc.sync.dma_start(
                            outd[b, dci * P:(dci + 1) * P,
                                 h * HGRP * TT:(h + 1) * HGRP * TT],
                            osb[:])
    return nc


def _run(inputs, trace=False):
    import concourse.bacc as bacc
    from concourse.bass_utils import run_bass_kernel_spmd

    hidden = np.ascontiguousarray(np.asarray(inputs["hidden"], dtype=np.float32))
    duration = np.asarray(inputs["duration"], dtype=np.float32)

    cbt, s, klo, khi, need_m, msub = _host_prep(duration)

    nc = bacc.Bacc("TRN2", target_bir_lowering=False, debug=False,
                   enable_asserts=False, num_devices=NCORES)
    _build(nc, klo, khi, need_m, s)
    nc.compile()

    in_maps = []
    for i in range(NCORES):
        in_maps.append({
            "hidden": hidden[i * BPC:(i + 1) * BPC],
            "cbt": np.ascontiguousarray(cbt[i * BPC:(i + 1) * BPC]),
            "msub": np.ascontiguousarray(msub[i * BPC:(i + 1) * BPC]),
        })
    res = run_bass_kernel_spmd(nc, in_maps, core_ids=list(range(NCORES)),
                               trace=trace)
    out = np.concatenate([res.results[i]["out"] for i in range(NCORES)], axis=0)
    return out, res


def kernel(**inputs) -> np.ndarray:
    out, _ = _run(inputs, trace=False)
    return out
